# revision 1
# baseline (speedup 1.0000x reference)
"""2-layer GAT (100000 nodes, 32 neighbors) on 8 trn2 NeuronCores.

Strategy (SPMD, one Bass program for all 8 cores):
  - Nodes are sharded 8 ways (12500/core) for the expensive per-edge work;
    the small weight matrices are replicated (fused on the host into one
    rhs per layer: [W | W@A1blk | W@A2blk] so one PE matmul per 128-node
    chunk emits h, s1=a1.h and s2=a2.h together).
  - Each core redundantly builds the full layer-1 node table
    [N,128] rows=[h1(64)|s1(8)|s2(8)|pad] from the replicated transposed
    features (cheap matmul, avoids a collective).
  - Neighbor gathers: per 128-destination-node tile, 33 indirect DMAs
    (one per neighbor slot; slot 0 gathers the destination row itself so
    its s1 arrives with the same gather).
  - Softmax attention (leaky_relu -> exp -> normalize -> weighted sum)
    runs on DVE/ACT with node-per-partition layout; ELU on ACT+DVE.
  - Layer-1 outputs are PE-transposed and AllGathered across the 8 cores
    (x2^T, 25.6 MB) so every core can build the full layer-2 table
    [N,144] rows=[h2(128)|s1(8)|s2(8)].
  - Layer-2 repeats the gather+attention, then head-mean + softmax.
Output: per-core [12500,16] shard, concatenated on the host.
"""
import sys

if '/opt/trn_rl_repo' not in sys.path:
    sys.path.insert(0, '/opt/trn_rl_repo')

import numpy as np
import concourse.bass as bass
import concourse.bacc as bacc
import concourse.mybir as mybir
from concourse.tile import TileContext
from concourse.masks import make_identity

import jax
from jax.sharding import Mesh, PartitionSpec
from jax.experimental.shard_map import shard_map
from concourse.bass2jax import (_bass_exec_p, install_neuronx_cc_hook,
                                partition_id_tensor)

FP = mybir.dt.float32
AF = mybir.ActivationFunctionType
OP = mybir.AluOpType
AX = mybir.AxisListType

N_NODES = 100000
N_CORES = 8
D_NBR = 32
K1, F1 = 8, 8
K2, F2 = 8, 16
NEG_SLOPE = 0.01


def _build_gat(N=N_NODES, n_cores=N_CORES, D=D_NBR):
    S = N // n_cores
    n_tiles = (S + 127) // 128
    n_chunks = (N + 127) // 128
    R1, R2 = 128, 144                    # table row strides (f32 elems)
    H1, H2 = K1 * F1, K2 * F2            # 64, 128
    IN1 = 128
    NI = D + 1

    nc = bacc.Bacc("TRN2", target_bir_lowering=False, debug=False,
                   num_devices=n_cores)
    xT = nc.dram_tensor("xT", [IN1, N], FP, kind="ExternalInput").ap()
    rhs1 = nc.dram_tensor("rhs1", [IN1, H1 + 16], FP, kind="ExternalInput").ap()
    rhs2 = nc.dram_tensor("rhs2", [H1, H2 + 16], FP, kind="ExternalInput").ap()
    nbr = nc.dram_tensor("nbr", [n_tiles * 128, NI], mybir.dt.int32,
                         kind="ExternalInput").ap()
    out = nc.dram_tensor("out", [S, F2], FP, kind="ExternalOutput").ap()

    table1 = nc.dram_tensor("table1", [N, R1], FP).ap()
    table2 = nc.dram_tensor("table2", [N, R2], FP).ap()
    x2T_shard = nc.dram_tensor("x2T_shard", [H1, S], FP).ap()
    x2T_all = nc.dram_tensor("x2T_all", [n_cores * H1, S], FP,
                             addr_space="Shared").ap()
    x2T_bounce = nc.dram_tensor("x2T_bounce", [H1, S], FP).ap()

    with TileContext(nc) as tc:
        with tc.tile_pool(name="const", bufs=1) as cpool, \
             tc.tile_pool(name="tb", bufs=4) as tbp, \
             tc.tile_pool(name="att", bufs=2) as ap_, \
             tc.tile_pool(name="psum", bufs=2, space="PSUM") as pp:

            rt1 = cpool.tile([IN1, H1 + 16], FP)
            nc.sync.dma_start(out=rt1[:], in_=rhs1[:, :])
            rt2 = cpool.tile([H1, H2 + 16], FP)
            nc.sync.dma_start(out=rt2[:], in_=rhs2[:, :])
            ident = cpool.tile([128, 128], FP)
            make_identity(nc, ident[:])

            # phase T1: full layer-1 table via fused matmul
            for c in range(n_chunks):
                g0 = c * 128
                M = min(128, N - g0)
                lt = tbp.tile([IN1, 128], FP, name=f"t1l{c}", tag="t1l")
                nc.sync.dma_start(out=lt[:, :M], in_=xT[:, g0:g0 + M])
                ps = pp.tile([128, H1 + 16], FP, name=f"t1p{c}", tag="t1p",
                             space="PSUM")
                nc.tensor.matmul(out=ps[:M, :], lhsT=lt[:, :M], rhs=rt1[:],
                                 start=True, stop=True)
                row = tbp.tile([128, R1], FP, name=f"t1r{c}", tag="t1r")
                nc.vector.tensor_copy(out=row[:M, :H1 + 16], in_=ps[:M, :])
                nc.sync.dma_start(out=table1[g0:g0 + M, :], in_=row[:M, :])

            # phase A1: layer-1 attention over the core's shard
            for t in range(n_tiles):
                r0 = t * 128
                M = min(128, S - r0)
                it = ap_.tile([128, NI], mybir.dt.int32, name=f"a1i{t}",
                              tag="a1i")
                nc.sync.dma_start(out=it[:], in_=nbr[r0:r0 + 128, :])
                hg = ap_.tile([128, NI * R1], FP, name=f"a1g{t}", tag="a1g")
                hgv = hg[:].rearrange("p (n r) -> p n r", r=R1)
                for j in range(NI):
                    nc.gpsimd.indirect_dma_start(
                        out=hgv[:, j, :], out_offset=None, in_=table1[:],
                        in_offset=bass.IndirectOffsetOnAxis(
                            ap=it[:, j:j + 1], axis=0))
                e = ap_.tile([128, D * K1], FP, name=f"a1e{t}", tag="a1e")
                nc.vector.tensor_tensor(
                    out=e[:].rearrange("p (d k) -> p d k", k=K1),
                    in0=hgv[:, 1:, H1 + 8:H1 + 16],
                    in1=hgv[:, 0:1, H1:H1 + 8].to_broadcast([128, D, K1]),
                    op=OP.add)
                u = ap_.tile([128, D * K1], FP, name=f"a1u{t}", tag="a1u")
                nc.scalar.activation(out=u[:], in_=e[:], func=AF.Lrelu,
                                     alpha=NEG_SLOPE)
                nc.scalar.activation(out=u[:], in_=u[:], func=AF.Exp)
                z = ap_.tile([128, K1], FP, name=f"a1z{t}", tag="a1z")
                nc.vector.tensor_reduce(
                    out=z[:],
                    in_=u[:].rearrange("p (d k) -> p d k", k=K1)
                        .transpose([0, 2, 1]),
                    axis=AX.X, op=OP.add)
                rz = ap_.tile([128, K1], FP, name=f"a1rz{t}", tag="a1rz")
                nc.vector.reciprocal(out=rz[:], in_=z[:])
                tmp = ap_.tile([128, H1 * D], FP, name=f"a1t{t}", tag="a1t")
                h4 = hgv[:, 1:, 0:H1].rearrange("p d (k f) -> p d k f", f=F1) \
                    .transpose([0, 2, 3, 1])
                u4 = u[:].rearrange("p (d k) -> p d k", k=K1).unsqueeze(3) \
                    .to_broadcast([128, D, K1, F1]).transpose([0, 2, 3, 1])
                nc.vector.tensor_tensor(
                    out=tmp[:].rearrange("p (k f d) -> p k f d", f=F1, d=D),
                    in0=h4, in1=u4, op=OP.mult)
                s = ap_.tile([128, H1], FP, name=f"a1s{t}", tag="a1s")
                nc.vector.tensor_reduce(
                    out=s[:], in_=tmp[:].rearrange("p (kf d) -> p kf d", d=D),
                    axis=AX.X, op=OP.add)
                o = ap_.tile([128, H1], FP, name=f"a1o{t}", tag="a1o")
                nc.vector.tensor_tensor(
                    out=o[:].rearrange("p (k f) -> p k f", f=F1),
                    in0=s[:].rearrange("p (k f) -> p k f", f=F1),
                    in1=rz[:].unsqueeze(2).to_broadcast([128, K1, F1]),
                    op=OP.mult)
                # elu(o) = max(o, exp(min(o,0)) - 1)
                mn = ap_.tile([128, H1], FP, name=f"a1m{t}", tag="a1m")
                nc.vector.tensor_scalar_min(out=mn[:], in0=o[:], scalar1=0.0)
                nc.scalar.activation(out=mn[:], in_=mn[:], func=AF.Exp)
                x2 = ap_.tile([128, H1], FP, name=f"a1x{t}", tag="a1x")
                nc.vector.scalar_tensor_tensor(
                    out=x2[:], in0=mn[:], scalar=-1.0, in1=o[:],
                    op0=OP.add, op1=OP.max)
                pt = pp.tile([H1, 128], FP, name=f"a1pt{t}", tag="a1pt",
                             space="PSUM")
                nc.tensor.transpose(out=pt[:], in_=x2[:], identity=ident[:])
                xt = ap_.tile([H1, 128], FP, name=f"a1xt{t}", tag="a1xt")
                nc.vector.tensor_copy(out=xt[:], in_=pt[:])
                nc.sync.dma_start(out=x2T_shard[:, r0:r0 + M], in_=xt[:, :M])

            # phase AG: exchange x2^T
            nc.sync.dma_start(out=x2T_bounce[:, :], in_=x2T_shard[:, :])
            nc.gpsimd.collective_compute(
                "AllGather", OP.bypass,
                replica_groups=[list(range(n_cores))],
                ins=[x2T_bounce.opt()], outs=[x2T_all.opt()])

            # phase T2: full layer-2 table
            for c in range(n_chunks):
                g0 = c * 128
                M = min(128, N - g0)
                lt2 = tbp.tile([H1, 128], FP, name=f"t2l{c}", tag="t2l")
                r_a, i_a = divmod(g0, S)
                n1 = min(M, S - i_a)
                nc.sync.dma_start(
                    out=lt2[:, 0:n1],
                    in_=x2T_all[r_a * H1:(r_a + 1) * H1, i_a:i_a + n1])
                if n1 < M:
                    r_b = r_a + 1
                    nc.sync.dma_start(
                        out=lt2[:, n1:M],
                        in_=x2T_all[r_b * H1:(r_b + 1) * H1, 0:M - n1])
                ps2 = pp.tile([128, R2], FP, name=f"t2p{c}", tag="t2p",
                              space="PSUM")
                nc.tensor.matmul(out=ps2[:M, :], lhsT=lt2[:, :M], rhs=rt2[:],
                                 start=True, stop=True)
                row2 = tbp.tile([128, R2], FP, name=f"t2r{c}", tag="t2r")
                nc.vector.tensor_copy(out=row2[:M, :], in_=ps2[:M, :])
                nc.sync.dma_start(out=table2[g0:g0 + M, :], in_=row2[:M, :])

            # phase A2: layer-2 attention + head mean + softmax
            for t in range(n_tiles):
                r0 = t * 128
                M = min(128, S - r0)
                it2 = ap_.tile([128, NI], mybir.dt.int32, name=f"a2i{t}",
                               tag="a2i")
                nc.sync.dma_start(out=it2[:], in_=nbr[r0:r0 + 128, :])
                hg2 = ap_.tile([128, NI * R2], FP, name=f"a2g{t}", tag="a2g")
                hg2v = hg2[:].rearrange("p (n r) -> p n r", r=R2)
                for j in range(NI):
                    nc.gpsimd.indirect_dma_start(
                        out=hg2v[:, j, :], out_offset=None, in_=table2[:],
                        in_offset=bass.IndirectOffsetOnAxis(
                            ap=it2[:, j:j + 1], axis=0))
                e2 = ap_.tile([128, D * K2], FP, name=f"a2e{t}", tag="a2e")
                nc.vector.tensor_tensor(
                    out=e2[:].rearrange("p (d k) -> p d k", k=K2),
                    in0=hg2v[:, 1:, H2 + 8:H2 + 16],
                    in1=hg2v[:, 0:1, H2:H2 + 8].to_broadcast([128, D, K2]),
                    op=OP.add)
                u2 = ap_.tile([128, D * K2], FP, name=f"a2u{t}", tag="a2u")
                nc.scalar.activation(out=u2[:], in_=e2[:], func=AF.Lrelu,
                                     alpha=NEG_SLOPE)
                nc.scalar.activation(out=u2[:], in_=u2[:], func=AF.Exp)
                z2 = ap_.tile([128, K2], FP, name=f"a2z{t}", tag="a2z")
                nc.vector.tensor_reduce(
                    out=z2[:],
                    in_=u2[:].rearrange("p (d k) -> p d k", k=K2)
                        .transpose([0, 2, 1]),
                    axis=AX.X, op=OP.add)
                rz2 = ap_.tile([128, K2], FP, name=f"a2rz{t}", tag="a2rz")
                nc.vector.reciprocal(out=rz2[:], in_=z2[:])
                tmp2 = ap_.tile([128, H2 * D], FP, name=f"a2t{t}", tag="a2t")
                h24 = hg2v[:, 1:, 0:H2].rearrange("p d (k f) -> p d k f",
                                                  f=F2).transpose([0, 2, 3, 1])
                u24 = u2[:].rearrange("p (d k) -> p d k", k=K2).unsqueeze(3) \
                    .to_broadcast([128, D, K2, F2]).transpose([0, 2, 3, 1])
                nc.vector.tensor_tensor(
                    out=tmp2[:].rearrange("p (k f d) -> p k f d", f=F2, d=D),
                    in0=h24, in1=u24, op=OP.mult)
                s2t = ap_.tile([128, H2], FP, name=f"a2s{t}", tag="a2s")
                nc.vector.tensor_reduce(
                    out=s2t[:],
                    in_=tmp2[:].rearrange("p (kf d) -> p kf d", d=D),
                    axis=AX.X, op=OP.add)
                o2 = ap_.tile([128, H2], FP, name=f"a2o{t}", tag="a2o")
                nc.vector.tensor_tensor(
                    out=o2[:].rearrange("p (k f) -> p k f", f=F2),
                    in0=s2t[:].rearrange("p (k f) -> p k f", f=F2),
                    in1=rz2[:].unsqueeze(2).to_broadcast([128, K2, F2]),
                    op=OP.mult)
                mo = ap_.tile([128, F2], FP, name=f"a2mo{t}", tag="a2mo")
                nc.vector.tensor_reduce(
                    out=mo[:],
                    in_=o2[:].rearrange("p (k f) -> p k f", f=F2)
                        .transpose([0, 2, 1]),
                    axis=AX.X, op=OP.add)
                u3 = ap_.tile([128, F2], FP, name=f"a2u3{t}", tag="a2u3")
                z3 = ap_.tile([128, 1], FP, name=f"a2z3{t}", tag="a2z3")
                nc.scalar.activation(out=u3[:], in_=mo[:], func=AF.Exp,
                                     scale=1.0 / K2, accum_out=z3[:])
                rz3 = ap_.tile([128, 1], FP, name=f"a2rz3{t}", tag="a2rz3")
                nc.vector.reciprocal(out=rz3[:], in_=z3[:])
                ot = ap_.tile([128, F2], FP, name=f"a2ot{t}", tag="a2ot")
                nc.vector.tensor_tensor(
                    out=ot[:], in0=u3[:],
                    in1=rz3[:].to_broadcast([128, F2]), op=OP.mult)
                nc.sync.dma_start(out=out[r0:r0 + M, :], in_=ot[:M, :])

    nc.finalize()
    return nc


class _SpmdRunner:
    """jit-once SPMD executor over the 8 axon NeuronCores."""

    def __init__(self, nc, n_cores):
        install_neuronx_cc_hook()
        self.nc, self.n_cores = nc, n_cores
        partition_name = (nc.partition_id_tensor.name
                          if nc.partition_id_tensor else None)
        in_names, out_names, out_avals, zero_outs = [], [], [], []
        for alloc in nc.m.functions[0].allocations:
            if not isinstance(alloc, mybir.MemoryLocationSet):
                continue
            name = alloc.memorylocations[0].name
            if alloc.kind == "ExternalInput":
                if name != partition_name:
                    in_names.append(name)
            elif alloc.kind == "ExternalOutput":
                out_names.append(name)
                shape = tuple(alloc.tensor_shape)
                dtype = mybir.dt.np(alloc.dtype)
                out_avals.append(jax.core.ShapedArray(shape, dtype))
                zero_outs.append(np.zeros(shape, dtype))
        self.in_names, self.out_names = in_names, out_names
        self.out_avals, self.zero_outs = out_avals, zero_outs
        all_in_names = in_names + out_names
        if partition_name is not None:
            all_in_names.append(partition_name)

        def _body(*args):
            operands = list(args)
            if partition_name is not None:
                operands.append(partition_id_tensor())
            return tuple(_bass_exec_p.bind(
                *operands, out_avals=tuple(out_avals),
                in_names=tuple(all_in_names), out_names=tuple(out_names),
                lowering_input_output_aliases=(),
                sim_require_finite=True, sim_require_nnan=True, nc=nc))

        devices = jax.devices()[:n_cores]
        self.mesh = Mesh(np.asarray(devices), ("core",))
        n_params, n_outs = len(in_names), len(out_avals)
        in_specs = (PartitionSpec("core"),) * (n_params + n_outs)
        out_specs = (PartitionSpec("core"),) * n_outs
        self.fn = jax.jit(
            shard_map(_body, mesh=self.mesh, in_specs=in_specs,
                      out_specs=out_specs, check_rep=False),
            keep_unused=True)
        self.sharding = jax.sharding.NamedSharding(self.mesh,
                                                   PartitionSpec("core"))

    def run(self, in_maps):
        per_core = [[np.asarray(m[n]) for n in self.in_names] for m in in_maps]
        concat = [np.concatenate([per_core[c][i] for c in range(self.n_cores)],
                                 axis=0) for i in range(len(self.in_names))]
        zeros = [np.zeros((self.n_cores * z.shape[0], *z.shape[1:]), z.dtype)
                 for z in self.zero_outs]
        dev = [jax.device_put(a, self.sharding) for a in concat + zeros]
        outs = self.fn(*dev)
        jax.block_until_ready(outs)
        res = []
        for c in range(self.n_cores):
            res.append({name: np.asarray(outs[i]).reshape(
                self.n_cores, *self.out_avals[i].shape)[c]
                for i, name in enumerate(self.out_names)})
        return res


def _host_prep(node_features, neighbors, W1, a1_1, a2_1, W2, a1_2, a2_2):
    N = node_features.shape[0]
    S = N // N_CORES
    n_tiles = (S + 127) // 128

    def blk(a, k, f):
        A = np.zeros((k * f, k), np.float32)
        for kk in range(k):
            A[kk * f:(kk + 1) * f, kk] = a[kk]
        return A

    rhs1 = np.concatenate(
        [W1, W1 @ blk(a1_1, K1, F1), W1 @ blk(a2_1, K1, F1)],
        axis=1).astype(np.float32)
    rhs2 = np.concatenate(
        [W2, W2 @ blk(a1_2, K2, F2), W2 @ blk(a2_2, K2, F2)],
        axis=1).astype(np.float32)
    xT = np.ascontiguousarray(node_features.T).astype(np.float32)

    in_maps = []
    for r in range(N_CORES):
        ids = np.arange(r * S, (r + 1) * S, dtype=np.int32)
        nb = np.concatenate(
            [ids[:, None], neighbors[r * S:(r + 1) * S].astype(np.int32)],
            axis=1)
        pad = n_tiles * 128 - S
        if pad:
            nb = np.concatenate([nb, np.zeros((pad, D_NBR + 1), np.int32)],
                                axis=0)
        in_maps.append({'xT': xT, 'rhs1': rhs1, 'rhs2': rhs2, 'nbr': nb})
    return in_maps


_RUNNER = None


def _get_runner():
    global _RUNNER
    if _RUNNER is None:
        nc = _build_gat()
        _RUNNER = _SpmdRunner(nc, N_CORES)
    return _RUNNER


def kernel(node_features, neighbors, W1, a1_1, a2_1, W2, a1_2, a2_2):
    node_features = np.asarray(node_features, dtype=np.float32)
    neighbors = np.asarray(neighbors)
    runner = _get_runner()
    in_maps = _host_prep(node_features, neighbors,
                         np.asarray(W1, np.float32),
                         np.asarray(a1_1, np.float32),
                         np.asarray(a2_1, np.float32),
                         np.asarray(W2, np.float32),
                         np.asarray(a1_2, np.float32),
                         np.asarray(a2_2, np.float32))
    res = runner.run(in_maps)
    return np.concatenate([res[c]['out'] for c in range(N_CORES)], axis=0)

